# revision 40
# baseline (speedup 1.0000x reference)
import numpy as np

import concourse.bass as bass
import concourse.tile as tile
from concourse import bacc, mybir
from concourse.bass_utils import run_bass_kernel_spmd

B_FULL = 16384
D = 2048
NUM_CLASS = 1000
N_CORES = 8
B_CORE = B_FULL // N_CORES
P = 128
N_TILES = B_CORE // P
F_CHUNKS = D // P
CHUNK = 4
N_CHUNKS = N_TILES // CHUNK
ND_WEIGHT = 1.0

_PROG = None


def _build_program(debug=False):
    nc = bacc.Bacc("TRN2", target_bir_lowering=False, debug=debug,
                   num_devices=N_CORES, num_swdge_queues=1)

    F8 = mybir.dt.float8e4
    FT = mybir.dt.float32
    BF = mybir.dt.bfloat16
    Alu = mybir.AluOpType
    Act = mybir.ActivationFunctionType

    s_sm = nc.dram_tensor("s_sm", [N_CHUNKS, P, CHUNK, D], F8,
                          kind="ExternalInput").ap()
    sT = nc.dram_tensor("sT", [N_CHUNKS, P, CHUNK, B_CORE], F8,
                        kind="ExternalInput").ap()
    tT = nc.dram_tensor("tT", [N_CHUNKS, P, CHUNK, B_CORE], F8,
                        kind="ExternalInput").ap()
    T8 = nc.dram_tensor("T_EMB", [NUM_CLASS, D], F8,
                        kind="ExternalInput").ap()
    lab32 = nc.dram_tensor("lab32", [P, N_TILES], mybir.dt.int32,
                           kind="ExternalInput").ap()
    ident = nc.dram_tensor("ident", [P, P], FT, kind="ExternalInput").ap()
    out_ap = nc.dram_tensor("out", [1, 1], FT, kind="ExternalOutput").ap()

    with tile.TileContext(nc) as tc:
        with (
            tc.tile_pool(name="smio", bufs=4) as smio,
            tc.tile_pool(name="stio", bufs=4) as stio,
            tc.tile_pool(name="ttio", bufs=4) as ttio,
            tc.tile_pool(name="dump", bufs=4) as dump,
            tc.tile_pool(name="xdump", bufs=4) as xdump,
            tc.tile_pool(name="stats", bufs=8) as stats,
            tc.tile_pool(name="persist", bufs=1) as persist,
            tc.tile_pool(name="psum", bufs=1, space="PSUM") as psum_pool,
        ):
            lab32_sb = persist.tile([P, N_TILES], mybir.dt.int32)
            nc.sync.dma_start(out=lab32_sb[:], in_=lab32)
            ident_sb = persist.tile([P, P], FT)
            nc.scalar.dma_start(out=ident_sb[:], in_=ident)

            g_all = persist.tile([P, N_TILES, D], F8)
            gis = []
            for t in range(N_TILES):
                gis.append(nc.gpsimd.indirect_dma_start(
                    out=g_all[:, t, :], out_offset=None, in_=T8[:],
                    in_offset=bass.IndirectOffsetOnAxis(
                        ap=lab32_sb[:, t:t + 1], axis=0),
                ))

            sm_tiles = [None] * N_CHUNKS
            sT_tiles = [None] * N_CHUNKS
            tT_tiles = [None] * N_CHUNKS
            for c in range(N_CHUNKS):
                sm_tiles[c] = smio.tile([P, CHUNK, D], F8, tag="sm",
                                        name=f"sm{c}")
                sT_tiles[c] = stio.tile([P, CHUNK, B_CORE], F8, tag="sT",
                                        name=f"sTt{c}")
                tT_tiles[c] = ttio.tile([P, CHUNK, B_CORE], F8, tag="tT",
                                        name=f"tTt{c}")
            plan = [
                ("sync", s_sm, sm_tiles, 0), ("scalar", s_sm, sm_tiles, 1),
                ("sync", s_sm, sm_tiles, 2), ("scalar", s_sm, sm_tiles, 3),
                ("sync", sT, sT_tiles, 1), ("scalar", sT, sT_tiles, 0),
                ("sync", sT, sT_tiles, 3), ("scalar", sT, sT_tiles, 2),
                ("sync", tT, tT_tiles, 0), ("scalar", tT, tT_tiles, 1),
                ("sync", tT, tT_tiles, 2), ("scalar", tT, tT_tiles, 3),
            ]
            ring = {"sync": nc.sync, "scalar": nc.scalar}
            for eng_name, srct, dst_tiles, c in plan:
                di = ring[eng_name].dma_start(out=dst_tiles[c][:],
                                              in_=srct[c])
                if srct is sT and c in (0, 1):
                    tile.add_dep_helper(di.ins, gis[2].ins, sync=True,
                                        reason="sT after gather tile 2")
                if srct is sT and c in (2, 3):
                    tile.add_dep_helper(di.ins, gis[6].ins, sync=True,
                                        reason="sT tail after gather tile 6")
                if srct is tT and eng_name == "sync":
                    tile.add_dep_helper(di.ins, gis[11].ins, sync=True,
                                        reason="tT after gather tile 11")

            dots_a = persist.tile([P, N_TILES], FT)
            s2a = persist.tile([P, N_TILES], FT)
            t2a = persist.tile([P, N_TILES], FT)
            g2a = persist.tile([P, N_TILES], FT)

            BANK = 512
            WAVE = 2

            def emit_dots(trange):
                for t in trange:
                    c, j = divmod(t, CHUNK)
                    s_v = sm_tiles[c][:, j, :]
                    g_v = g_all[:, t, :]
                    d0 = dump.tile([P, D], BF, tag="d0", name=f"d0_{t}")
                    nc.vector.scalar_tensor_tensor(
                        out=d0[:], in0=g_v, scalar=1.0, in1=s_v,
                        op0=Alu.mult, op1=Alu.mult,
                        accum_out=dots_a[:, t:t + 1])
                    d1 = dump.tile([P, D], BF, tag="d1", name=f"d1_{t}")
                    nc.scalar.activation(out=d1[:], in_=g_v, func=Act.Square,
                                         accum_out=g2a[:, t:t + 1])

            def emit_pe_stat(xT_tiles, which):
                for wv in range(N_TILES // WAVE):
                    ps = psum_pool.tile([P, WAVE * BANK], FT,
                                        tag=f"ps{wv % 4}",
                                        name=f"ps_{which}_{wv}")
                    for f in range(F_CHUNKS):
                        c, j = divmod(f, CHUNK)
                        for k in range(WAVE):
                            b = wv * WAVE + k
                            w = xT_tiles[c][:, j, b * P:(b + 1) * P]
                            nc.tensor.matmul(
                                out=ps[:, k * BANK:k * BANK + P],
                                lhsT=w, rhs=w,
                                start=(f == 0), stop=(f == F_CHUNKS - 1))
                    yield wv, ps

            def emit_extracts(ps, xa, wv):
                for k in range(WAVE):
                    b = wv * WAVE + k
                    xd = xdump.tile([P, P], FT, tag="xd",
                                    name=f"xd{wv}_{k}")
                    nc.vector.scalar_tensor_tensor(
                        out=xd[:], in0=ps[:, k * BANK:k * BANK + P],
                        scalar=1.0, in1=ident_sb[:],
                        op0=Alu.mult, op1=Alu.mult,
                        accum_out=xa[:, b:b + 1])

            acc = stats.tile([P, N_TILES], FT, tag="acc")

            def emit_combine(h):
                sl = slice(8 * h, 8 * (h + 1))
                m2 = stats.tile([P, 8], FT, tag="m2", name=f"m2_{h}")
                nc.vector.tensor_tensor(out=m2[:], in0=s2a[:, sl],
                                        in1=t2a[:, sl], op=Alu.max)
                p2 = stats.tile([P, 8], FT, tag="p2", name=f"p2_{h}")
                nc.vector.tensor_tensor(out=p2[:], in0=m2[:],
                                        in1=g2a[:, sl], op=Alu.mult)
                rnorm = stats.tile([P, 8], FT, tag="rnorm", name=f"rn_{h}")
                nc.scalar.activation(out=rnorm[:], in_=p2[:], func=Act.Sqrt)
                rs = stats.tile([P, 8], FT, tag="rs", name=f"rs_{h}")
                nc.vector.reciprocal(out=rs[:], in_=rnorm[:])
                nc.vector.tensor_tensor(out=acc[:, sl], in0=dots_a[:, sl],
                                        in1=rs[:], op=Alu.mult)

            s2_waves = emit_pe_stat(sT_tiles, 0)
            t2_waves = emit_pe_stat(tT_tiles, 1)
            emit_dots(range(8))
            for t in range(8, 12):
                emit_dots([t])
                for wv, ps in [next(s2_waves), next(s2_waves)]:
                    emit_extracts(ps, s2a, wv)
            for t in range(12, 16):
                emit_dots([t])
                for wv, ps in [next(t2_waves), next(t2_waves)]:
                    emit_extracts(ps, t2a, wv)
                if t == 13:
                    emit_combine(0)

            emit_combine(1)

            rsum = persist.tile([P, 1], FT)
            nc.vector.tensor_reduce(out=rsum[:], in_=acc[:],
                                    axis=mybir.AxisListType.X, op=Alu.add)
            ones = persist.tile([P, 1], FT)
            nc.vector.memset(ones[:], 1.0)
            total = psum_pool.tile([1, 1], FT, tag="ps0")
            nc.tensor.matmul(out=total[:], lhsT=rsum[:], rhs=ones[:],
                             start=True, stop=True)
            res = persist.tile([1, 1], FT)
            nc.scalar.activation(out=res[:], in_=total[:], func=Act.Copy,
                                 bias=float(B_CORE) * ND_WEIGHT / B_FULL,
                                 scale=-ND_WEIGHT / B_FULL)
            nc.sync.dma_start(out=out_ap[:], in_=res[:])

    nc.compile()
    return nc


def _get_program():
    global _PROG
    if _PROG is None:
        _PROG = _build_program()
    return _PROG


def _chunked(x):
    n, cols = x.shape
    return np.ascontiguousarray(
        x.reshape(N_CHUNKS, CHUNK, P, cols).transpose(0, 2, 1, 3))


def _make_in_maps(s_emb, t_emb, T_EMB, labels):
    import ml_dtypes
    FP8 = ml_dtypes.float8_e4m3fn
    s_emb = np.asarray(s_emb, dtype=np.float32)
    t_emb = np.asarray(t_emb, dtype=np.float32)
    T8 = np.ascontiguousarray(
        np.asarray(T_EMB, dtype=np.float32).astype(FP8))
    labels_i = np.asarray(labels).astype(np.int64)
    ident = np.eye(P, dtype=np.float32)
    in_maps = []
    for i in range(N_CORES):
        lo, hi = i * B_CORE, (i + 1) * B_CORE
        s8 = s_emb[lo:hi].astype(FP8)
        t8 = t_emb[lo:hi].astype(FP8)
        lab_core = labels_i[lo:hi]
        lab32 = lab_core.reshape(N_TILES, P).T
        in_maps.append({
            "lab32": np.ascontiguousarray(lab32.astype(np.int32)),
            "s_sm": _chunked(s8),
            "sT": _chunked(np.ascontiguousarray(s8.T)),
            "tT": _chunked(np.ascontiguousarray(t8.T)),
            "T_EMB": T8,
            "ident": ident,
        })
    return in_maps


def run(s_emb, t_emb, T_EMB, labels, trace=False, **spmd_kwargs):
    nc = _get_program()
    in_maps = _make_in_maps(s_emb, t_emb, T_EMB, labels)
    res = run_bass_kernel_spmd(nc, in_maps, core_ids=list(range(N_CORES)),
                               trace=trace, **spmd_kwargs)
    partials = [res.results[i]["out"][0, 0] for i in range(N_CORES)]
    loss = np.array(np.sum(np.asarray(partials, dtype=np.float64)),
                    dtype=np.float32)
    return loss, res


def kernel(s_emb, t_emb, T_EMB, labels):
    loss, _ = run(s_emb, t_emb, T_EMB, labels)
    return loss
